# revision 6
# baseline (speedup 1.0000x reference)
"""GNN message-passing (3x GraphConv + mean-pool + FC + softmax, graph 0 only)
on 8 Trainium2 NeuronCores.

Strategy (v2)
-------------
Nodes are partitioned across the 8 cores (interleaved by in-degree rank so
per-core work is balanced). Only ``probs[0]`` is returned, so layers 2/3 are
pruned to the exact 1-hop/2-hop in-neighborhoods of graph-0 nodes, and layer
1 to the 3-hop set T1.

Layer 1 gathers **raw bf16 x-rows** (256B granules) straight from a
replicated, host-staged x table with the gather's 16-bit transpose mode, so
the slot grids arrive feature-major: the segment-sum is a free-dim tree fold
on DVE, and ``W_rel1^T @ agg + W_root1^T @ x_own`` is two PSUM-accumulated
matmuls — no AllGather and no PE transposes anywhere in layer 1.

Layers 2/3 gather fp32 z-rows (``z = h @ W_rel``, 256B) from replicated
tables built by AllGather. The tables are laid out block-major so the
AllGather is split into sub-block collectives kicked as soon as the producing
chunks finish — the wire time hides behind the producer layer's compute.
Per-column slot caps are chosen per chunk by exact host-side optimization
(grid rows + dense one-hot overflow rows minimized); overflow edges are
folded in via one-hot matmuls.

Mean-pool partials are AllReduce-d; every core computes the FC + softmax.
"""

import os

import numpy as np

import concourse.bacc as bacc
import concourse.bass as bass
import concourse.mybir as mybir
import concourse.tile as tile
from concourse._compat import cdiv
from concourse.bass_utils import run_bass_kernel_spmd
from concourse.masks import make_identity

NCORES = 8
LO = 32768  # int16 gather index limit per table
F32 = mybir.dt.float32
BF16 = mybir.dt.bfloat16
I16 = mybir.dt.int16
AX = mybir.AluOpType
ACTF = mybir.ActivationFunctionType

NB1 = 4   # z1 AllGather sub-blocks
NB2 = 2   # z2 AllGather sub-blocks
SLOT_CAP = 104  # max slot rows per gather group
OV_PENALTY = 1.25  # extra engine cost of an overflow row vs a grid row


def _group_rank_desc(keys: np.ndarray) -> np.ndarray:
    return np.argsort(-keys, kind="stable")


def _block_splits(n_chunks: int, n_blocks: int, min_first: int = 0) -> list[int]:
    """Chunk indices where AG blocks end; last is n_chunks."""
    n_blocks = max(1, min(n_blocks, n_chunks))
    ends = [round(n_chunks * (b + 1) / n_blocks) for b in range(n_blocks)]
    ends = sorted(set(max(1, e) for e in ends))
    if min_first and ends[0] < min_first:
        ends[0] = min_first
        ends = sorted(set(ends))
    if ends[-1] != n_chunks:
        ends.append(n_chunks)
    return [e for e in ends if e <= n_chunks]


class Plan:
    pass


def build_plan(x, edge_index, batch):
    """All host-side index crunching. Returns a Plan with shared (SPMD)
    compile-time constants and per-core input arrays."""
    p = Plan()
    N, F = x.shape
    src = np.asarray(edge_index[0], dtype=np.int64)
    dst = np.asarray(edge_index[1], dtype=np.int64)
    batch = np.asarray(batch, dtype=np.int64)
    p.N, p.F = N, F

    # --- pruning sets -------------------------------------------------------
    in_T0 = batch == 0
    p.n0 = int(in_T0.sum())
    e3 = in_T0[dst]
    in_T2 = in_T0.copy()
    in_T2[src[e3]] = True
    e2 = in_T2[dst]
    in_T1 = in_T2.copy()
    in_T1[src[e2]] = True
    e1 = in_T1[dst]

    deg1 = np.bincount(dst[e1], minlength=N)
    deg2 = np.bincount(dst[e2], minlength=N)
    deg3 = np.bincount(dst[e3], minlength=N)

    # --- balanced interleaved node->core assignment (T1 only) ---------------
    nodes = np.arange(N)
    g0 = nodes[in_T0]
    g1 = nodes[in_T2 & ~in_T0]
    g2 = nodes[in_T1 & ~in_T2]
    g0 = g0[_group_rank_desc(deg3[g0])]
    g1 = g1[_group_rank_desc(deg2[g1])]
    g2 = g2[_group_rank_desc(deg1[g2])]
    seq = np.concatenate([g0, g1, g2])
    j = np.arange(len(seq))
    seq_core = j % NCORES
    n0_k = np.bincount(seq_core[: len(g0)], minlength=NCORES)
    n2_k = np.bincount(seq_core[: len(g0) + len(g1)], minlength=NCORES)
    n1_k = np.bincount(seq_core, minlength=NCORES)

    C3 = max(1, cdiv(int(n0_k.max()), 128))
    C2 = max(C3, cdiv(int(n2_k.max()) + 1, 128))
    C1 = max(C2, cdiv(int(n1_k.max()) + 3, 128))
    Z1, Z2 = C1 * 128, C2 * 128
    p.C = [C1, C2, C3]

    # --- z-table AG blocks (chunk splits) -----------------------------------
    cb1 = _block_splits(C1, NB1, min_first=C2)   # z1 blocks end at cb1[i]*128
    cb2 = _block_splits(C2, NB2)
    p.cb1, p.cb2 = cb1, cb2

    # Reserved hole positions (zero rows usable as gather padding):
    holes = sorted({cb1[0] * 128 - 1, Z1 - 1, Z2 - 1})
    assert holes[0] >= Z2 - 1 >= int(n2_k.max()), (holes, n2_k.max())
    p.holes = holes

    # --- per-core local positions -------------------------------------------
    # Positions 0..Z1-1 minus holes, filled in seq order per core.
    pos = np.full(N, -1, np.int64)
    node_core = np.full(N, -1, np.int64)
    avail = np.array([q for q in range(Z1) if q not in set(holes)])
    for k in range(NCORES):
        kn = seq[seq_core == k]
        assert len(kn) <= len(avail)
        pos[kn] = avail[: len(kn)]
        node_core[kn] = k
        assert n2_k[k] == 0 or pos[kn[n2_k[k] - 1]] < Z2
    p.pos, p.node_core = pos, node_core
    p.n0_k = n0_k

    # --- x table (replicated, bf16, identity order) -------------------------
    NT = N + 2
    p.NT = NT
    p.Z = [NT, Z1, Z2]

    # --- z-table block-major row maps ---------------------------------------
    def make_rowmap(Zl, cends):
        """pos -> table row; core-major (row = k*Zl + pos): the whole-table
        AllGather concatenates core shards (Shared tensors allow only one
        writer instruction, so no per-block sub-AllGathers)."""
        rowbase = np.arange(Zl, dtype=np.int64)
        rowmul = np.full(Zl, Zl, np.int64)
        return rowbase, rowmul

    rb1, rm1 = make_rowmap(Z1, cb1)
    rb2, rm2 = make_rowmap(Z2, cb2)
    p.T1_rows = NCORES * Z1
    p.T2_rows = NCORES * Z2
    assert p.T1_rows <= 2 * LO and p.T2_rows <= LO

    def z1row(k, q):
        return rb1[q] + k * rm1[q]

    def z2row(k, q):
        return rb2[q] + k * rm2[q]

    # pad rows (guaranteed zero by hole construction)
    p.pad = [
        (0, NT - 1 - LO),                                  # x table lo/hi
        (int(z1row(0, cb1[0] * 128 - 1)), int(z1row(NCORES - 1, Z1 - 1)) - LO),
        (int(z2row(NCORES - 1, Z2 - 1)), 0),               # z2: lo only
    ]
    assert 0 <= p.pad[1][0] < LO and 0 <= p.pad[1][1] < LO
    assert 0 <= p.pad[2][0] < LO

    # --- per-layer edge -> (srcrow, col, core) ------------------------------
    layers = []
    esets = [(e1, C1), (e2, C2), (e3, C3)]
    for li, (emask, Cl) in enumerate(esets):
        es, ed = src[emask], dst[emask]
        if li == 0:
            srcpos = es + 1
        elif li == 1:
            srcpos = z1row(node_core[es], pos[es])
        else:
            srcpos = z2row(node_core[es], pos[es])
        col = pos[ed]
        keep = col < Cl * 128
        assert keep.all()
        ecore = node_core[ed]
        tab_rows = [NT, p.T1_rows, p.T2_rows][li]
        split = tab_rows > LO
        hi_half = srcpos >= LO if split else np.zeros(len(es), bool)

        # per (core, chunk, half) per-col counts -> optimal shared caps
        Cl128 = Cl * 128
        cnt = np.zeros((2, NCORES, Cl128), np.int64)
        for h in (0, 1):
            for k in range(NCORES):
                m = (ecore == k) & (hi_half == bool(h))
                cnt[h, k] = np.bincount(col[m], minlength=Cl128)
        R = np.zeros((2, Cl), np.int64)
        OV = np.zeros((2, Cl), np.int64)
        for h in (0, 1):
            ch = cnt[h].reshape(NCORES, Cl, 128)
            for c in range(Cl):
                cc = ch[:, c, :]                      # [NCORES, 128]
                mx = int(cc.max())
                if mx == 0:
                    continue
                best = (mx, mx, 0)
                for cap in range(mx + 1):
                    ov = int(np.ceil(np.maximum(cc - cap, 0).sum(1) / 128.0)
                             .max())
                    cost = cap + OV_PENALTY * ov
                    if cost < best[0] - 1e-9:
                        best = (cost, cap, ov)
                _, R[h, c], OV[h, c] = best
        layers.append(dict(li=li, Cl=Cl, split=split, srcpos=srcpos, col=col,
                           ecore=ecore, hi=hi_half,
                           Rlo=R[0], Rhi=R[1], OVlo=OV[0], OVhi=OV[1]))
    p.layers = layers

    # --- gather grouping ----------------------------------------------------
    # L1 overflow rows get their own stream (different gather mode); L2/L3
    # overflow rows ride in the same stream as their group.
    for L in layers:
        groups = []
        cur, cur_rows = [], 0
        for c in range(L["Cl"]):
            r = int(L["Rlo"][c] + L["Rhi"][c])
            if L["li"] > 0:
                r += int(L["OVlo"][c] + L["OVhi"][c])
            if cur and cur_rows + r > SLOT_CAP:
                groups.append(cur)
                cur, cur_rows = [], 0
            cur.append(c)
            cur_rows += r
        if cur:
            groups.append(cur)
        L["groups"] = groups

    # --- idx stream layout (shared across cores) ----------------------------
    off = 0   # slot-rows of 128
    ovb = 0   # overflow block columns
    for L in layers:
        li = L["li"]
        L["lo_off"] = {}
        L["hi_off"] = {}
        L["ovlo_off"] = {}
        L["ovhi_off"] = {}
        L["ovlo_col"] = {}
        L["ovhi_col"] = {}
        if li == 0:
            for grp in L["groups"]:
                for c in grp:
                    L["lo_off"][c] = off
                    off += int(L["Rlo"][c])
                for c in grp:
                    L["hi_off"][c] = off
                    off += int(L["Rhi"][c])
            # separate ov stream: all ovlo rows then all ovhi rows
            L["ov_base"] = off
            for c in range(L["Cl"]):
                L["ovlo_off"][c] = off
                L["ovlo_col"][c] = ovb
                off += int(L["OVlo"][c])
                ovb += int(L["OVlo"][c])
            L["ovlo_rows"] = off - L["ov_base"]
            for c in range(L["Cl"]):
                L["ovhi_off"][c] = off
                L["ovhi_col"][c] = ovb
                off += int(L["OVhi"][c])
                ovb += int(L["OVhi"][c])
            L["ovhi_rows"] = off - L["ov_base"] - L["ovlo_rows"]
        else:
            for grp in L["groups"]:
                for c in grp:
                    L["lo_off"][c] = off
                    off += int(L["Rlo"][c])
                for c in grp:
                    L["ovlo_off"][c] = off
                    L["ovlo_col"][c] = ovb
                    off += int(L["OVlo"][c])
                    ovb += int(L["OVlo"][c])
                for c in grp:
                    L["hi_off"][c] = off
                    off += int(L["Rhi"][c])
                for c in grp:
                    L["ovhi_off"][c] = off
                    L["ovhi_col"][c] = ovb
                    off += int(L["OVhi"][c])
                    ovb += int(L["OVhi"][c])
    p.total_rows = off
    p.WTOT = off * 8
    p.NOVB = max(ovb, 1)

    # --- per-core idx / ovdst arrays ----------------------------------------
    p.idx = []
    p.ovdst = []
    for k in range(NCORES):
        flat = np.zeros(p.total_rows * 128, np.int16)
        ovd = np.zeros(p.NOVB * 128, np.float32)
        for L in layers:
            li = L["li"]
            pad_lo, pad_hi = p.pad[li]
            m = L["ecore"] == k
            es_pos = L["srcpos"][m]
            ecol = L["col"][m]
            ehalf = L["hi"][m]
            order = np.lexsort((es_pos, ecol, ehalf))
            sc = ecol[order]
            sh = ehalf[order]
            sp = es_pos[order]
            key = sh.astype(np.int64) * (1 << 32) + sc
            newgrp = np.r_[True, np.diff(key) != 0]
            starts = np.flatnonzero(newgrp)
            lens = np.diff(np.r_[starts, len(key)])
            rank = np.arange(len(key)) - np.repeat(starts, lens)
            chunk = sc // 128
            lane = sc % 128
            # fill padding defaults
            for c in range(L["Cl"]):
                for rr, offmap in [(int(L["Rlo"][c]) * 128, "lo_off"),
                                   (int(L["OVlo"][c]) * 128, "ovlo_off")]:
                    if rr:
                        o = L[offmap][c] * 128
                        flat[o:o + rr] = pad_lo
                for rr, offmap in [(int(L["Rhi"][c]) * 128, "hi_off"),
                                   (int(L["OVhi"][c]) * 128, "ovhi_off")]:
                    if rr:
                        o = L[offmap][c] * 128
                        flat[o:o + rr] = pad_hi
            base_lo = np.array([L["lo_off"].get(c, 0) for c in range(L["Cl"])])
            base_hi = np.array([L["hi_off"].get(c, 0) for c in range(L["Cl"])])
            caps = np.stack([L["Rlo"], L["Rhi"]])
            for half, base, padshift in [(False, base_lo, 0),
                                         (True, base_hi, LO)]:
                capv = caps[int(half)][chunk]
                hm = (sh == half) & (rank < capv)
                if hm.any():
                    sl = (base[chunk[hm]] + rank[hm]) * 128 + lane[hm]
                    flat[sl] = (sp[hm] - padshift).astype(np.int16)
                om = (sh == half) & (rank >= capv)
                if om.any():
                    oidx = np.flatnonzero(om)
                    och = chunk[oidx]
                    osort = np.argsort(och, kind="stable")
                    oo = oidx[osort]
                    cch = chunk[oo]
                    st = np.r_[True, np.diff(cch) != 0]
                    sts = np.flatnonzero(st)
                    ll = np.diff(np.r_[sts, len(cch)])
                    orank = np.arange(len(cch)) - np.repeat(sts, ll)
                    obase = np.array(
                        [L["ovlo_off" if not half else "ovhi_off"].get(c, 0)
                         for c in range(L["Cl"])])
                    cbase = np.array(
                        [L["ovlo_col" if not half else "ovhi_col"].get(c, 0)
                         for c in range(L["Cl"])])
                    slot = obase[cch] * 128 + orank
                    flat[slot] = (sp[oo] - padshift).astype(np.int16)
                    ovslot = cbase[cch] * 128 + orank
                    ovd[ovslot] = (sc[oo] % 128).astype(np.float32)
        wrapped = flat.reshape(-1, 16).T.copy()
        p.idx.append(np.tile(wrapped, (8, 1)))               # [128, WTOT]
        p.ovdst.append(np.ascontiguousarray(
            ovd.reshape(p.NOVB, 128).T))                     # [128, NOVB]

    # --- staged tensors -----------------------------------------------------
    import ml_dtypes
    bf = ml_dtypes.bfloat16
    x32 = np.asarray(x, np.float32)
    xtab = np.zeros((NT, F), bf)
    xtab[1:N + 1] = x32.astype(bf)
    p.x_tab = xtab                                           # replicated

    p.xTown = []
    for k in range(NCORES):
        xp = np.zeros((Z1, F), np.float32)
        kn = nodes[node_core == k]
        xp[pos[kn]] = x32[kn]
        p.xTown.append(np.ascontiguousarray(xp.T).astype(bf))  # [F, Z1] bf16

    p.mask = []
    for k in range(NCORES):
        msk = np.zeros(C3 * 128, np.float32)
        msk[:n0_k[k]] = 1.0
        p.mask.append(np.broadcast_to(msk, (64, C3 * 128)).astype(bf).copy())

    return p


# ----------------------------------------------------------------------------
# Device program
# ----------------------------------------------------------------------------

def build_program(p, W, skip_collectives=False, repeat=1):
    """Emit the Bass/Tile program for one core (SPMD)."""
    nc = bacc.Bacc("TRN2")
    F = p.F
    C1, C2, C3 = p.C
    Z1, Z2 = C1 * 128, C2 * 128

    x_tab_d = nc.dram_tensor("x_tab", [p.NT, F], BF16, kind="ExternalInput")
    xTown_d = nc.dram_tensor("xTown", [F, Z1], BF16, kind="ExternalInput")
    idx_d = nc.dram_tensor("idx", [128, p.WTOT], I16, kind="ExternalInput")
    mask_d = nc.dram_tensor("mask", [64, C3 * 128], BF16,
                            kind="ExternalInput")
    wrel1_d = nc.dram_tensor("bW_rel1", [F, 64], BF16, kind="ExternalInput")
    wroot1_d = nc.dram_tensor("bW_root1", [F, 64], BF16, kind="ExternalInput")
    w64_d = {}
    for nm in ["W_rel2", "W_root2", "W_rel3", "W_root3"]:
        w64_d[nm] = nc.dram_tensor("b" + nm, [64, 64], BF16,
                                   kind="ExternalInput")
    wfc_d = nc.dram_tensor("W_fc", [64, 10], F32, kind="ExternalInput")
    bfc_d = nc.dram_tensor("b_fc", [1, 10], F32, kind="ExternalInput")
    ovdst_d = nc.dram_tensor("ovdst", [128, p.NOVB], F32,
                             kind="ExternalInput")
    iota_d = nc.dram_tensor("iota", [128, 128], F32, kind="ExternalInput")
    out_d = nc.dram_tensor("probs", [1, 10], F32, kind="ExternalOutput")

    rg = [list(range(NCORES))]
    L1, L2, L3 = p.layers

    with tile.TileContext(nc) as tc:
        with (
            tc.tile_pool(name="const", bufs=1) as cpool,
            tc.tile_pool(name="persist", bufs=1) as ppool,
            tc.tile_pool(name="stream", bufs=3) as spool,
            tc.tile_pool(name="gather", bufs=2) as gpool,
            tc.tile_pool(name="psum", bufs=1, space="PSUM") as psum,
            tc.tile_pool(name="dram", bufs=1, space="DRAM") as dram,
        ):
            # ---- constants ----
            ident = cpool.tile([128, 128], F32)
            make_identity(nc, ident[:])
            wr1_s = cpool.tile([F, 64], BF16, tag="wr1")
            wo1_s = cpool.tile([F, 64], BF16, tag="wo1")
            nc.sync.dma_start(wr1_s[:], wrel1_d[:])
            nc.sync.dma_start(wo1_s[:], wroot1_d[:])
            w64 = {}
            for nm, d in w64_d.items():
                w64[nm] = cpool.tile([64, 64], BF16, tag=nm, name=nm)
                nc.sync.dma_start(w64[nm][:], d[:])
            wfc_s = cpool.tile([64, 10], F32, tag="wfc")
            nc.sync.dma_start(wfc_s[:], wfc_d[:])
            bfc_s = cpool.tile([1, 10], F32, tag="bfc")
            nc.sync.dma_start(bfc_s[:], bfc_d[:])
            mask_s = cpool.tile([64, C3 * 128], BF16, tag="mask")
            nc.sync.dma_start(mask_s[:], mask_d[:])
            idx_s = cpool.tile([128, p.WTOT], I16, tag="idx")
            nc.sync.dma_start(idx_s[:], idx_d[:])
            xTown_s = cpool.tile([F, Z1], BF16, tag="xTown")
            nc.sync.dma_start(xTown_s[:], xTown_d[:])
            ovdst_s = cpool.tile([128, p.NOVB], F32, tag="ovdst")
            nc.sync.dma_start(ovdst_s[:], ovdst_d[:])
            iota_s = cpool.tile([128, 128], F32, tag="iota")
            nc.sync.dma_start(iota_s[:], iota_d[:])

            # ---- persistent feature-major activations ----
            hT = [ppool.tile([64, Z1], BF16, tag="h1T", name="h1T"),
                  ppool.tile([64, Z2], BF16, tag="h2T", name="h2T"),
                  ppool.tile([64, C3 * 128], BF16, tag="h3T", name="h3T")]

            dbg_stage = int(os.environ.get("GNN_DEBUG_STAGE", "3"))
            for _rep in range(repeat):
                z_own = [None,
                         dram.tile([Z1, 64], F32, name="z1o_%d" % _rep),
                         dram.tile([Z2, 64], F32, name="z2o_%d" % _rep)]
                z_tab = [None,
                         dram.tile([p.T1_rows, 64], F32, addr_space="Shared",
                                   name="z1t_%d" % _rep),
                         dram.tile([p.T2_rows, 64], F32, addr_space="Shared",
                                   name="z2t_%d" % _rep)]
                pool_in = dram.tile([64, 1], F32, name="pool_in_%d" % _rep)
                pool_out = dram.tile([64, 1], F32, addr_space="Shared",
                                     name="pool_out_%d" % _rep)

                def gath(g_ap, tab_ap, o, rows, elem, transpose):
                    nc.gpsimd.dma_gather(
                        g_ap, tab_ap, idx_s[:, o * 8:(o + rows) * 8],
                        rows * 128, rows * 128, elem,
                        transpose=transpose, single_packet=False)

                def store_z(li, c):
                    """z[li+1] chunk c (node-major) -> z_own[li+1]."""
                    wrel = w64["W_rel2"] if li == 0 else w64["W_rel3"]
                    sl = slice(c * 128, (c + 1) * 128)
                    zp = psum.tile([128, 64], F32, tag="zn", bufs=2)
                    nc.tensor.matmul(zp[:], lhsT=hT[li][:, sl], rhs=wrel[:],
                                     start=True, stop=True)
                    zs = spool.tile([128, 64], F32, tag="zns")
                    nc.scalar.activation(zs[:], zp[:], ACTF.Copy)
                    nc.sync.dma_start(z_own[li + 1][sl, :], zs[:])

                def kick_ag(li, c0, c1):
                    if skip_collectives:
                        return
                    # block-major: out rows [8*c0*128, 8*c1*128)
                    nc.gpsimd.collective_compute(
                        "AllGather", AX.bypass, replica_groups=rg,
                        ins=[z_own[li][c0 * 128:c1 * 128, :].opt()],
                        outs=[z_tab[li][NCORES * c0 * 128:
                                        NCORES * c1 * 128, :].opt()])

                # ============ Layer 1: bf16 x-row gathers ============
                # overflow rows (one stream, non-transpose -> edge-major)
                novl, novh = L1["ovlo_rows"], L1["ovhi_rows"]
                gov = None
                if novl + novh:
                    gov = ppool.tile([128, novl + novh, F], BF16,
                                     tag="gov", name="gov_%d" % _rep)
                    if novl:
                        gath(gov[:, 0:novl, :], x_tab_d[0:LO, :],
                             L1["ov_base"], novl, F, False)
                    if novh:
                        gath(gov[:, novl:, :], x_tab_d[LO:p.NT, :],
                             L1["ov_base"] + novl, novh, F, False)

                for grp in L1["groups"]:
                    rows_lo = sum(int(L1["Rlo"][c]) for c in grp)
                    rows_hi = sum(int(L1["Rhi"][c]) for c in grp)
                    rows = rows_lo + rows_hi
                    g = None
                    if rows:
                        g = gpool.tile([128, 1, rows * 128], BF16, tag="G1",
                                       name="G1")
                        if rows_lo:
                            gath(g[:, :, 0:rows_lo * 128], x_tab_d[0:LO, :],
                                 L1["lo_off"][grp[0]], rows_lo, F, True)
                        if rows_hi:
                            gath(g[:, :, rows_lo * 128:rows * 128],
                                 x_tab_d[LO:p.NT, :],
                                 L1["hi_off"][grp[0]], rows_hi, F, True)

                    for c in grp:
                        sl = slice(c * 128, (c + 1) * 128)
                        rlo = int(L1["Rlo"][c])
                        rhi = int(L1["Rhi"][c])

                        def fold1(a, r):
                            k = r
                            while k > 1:
                                m = (k + 1) // 2
                                cnt = k - m
                                nc.vector.tensor_tensor(
                                    out=g[:, 0, a * 128:(a + cnt) * 128],
                                    in0=g[:, 0, a * 128:(a + cnt) * 128],
                                    in1=g[:, 0, (a + m) * 128:(a + k) * 128],
                                    op=AX.add)
                                k = m

                        la = L1["lo_off"][c] - L1["lo_off"][grp[0]]
                        ha = rows_lo + L1["hi_off"][c] - L1["hi_off"][grp[0]]
                        if rlo:
                            fold1(la, rlo)
                        if rhi:
                            fold1(ha, rhi)
                        if rlo and rhi:
                            nc.vector.tensor_tensor(
                                out=g[:, 0, la * 128:(la + 1) * 128],
                                in0=g[:, 0, la * 128:(la + 1) * 128],
                                in1=g[:, 0, ha * 128:(ha + 1) * 128],
                                op=AX.add)
                        agg = None
                        if rlo:
                            agg = g[:, 0, la * 128:(la + 1) * 128]
                        elif rhi:
                            agg = g[:, 0, ha * 128:(ha + 1) * 128]

                        # overflow one-hot blocks -> PSUM [F, 128]
                        ovblk = []
                        for b in range(int(L1["OVlo"][c])):
                            ovblk.append((L1["ovlo_off"][c] - L1["ov_base"]
                                          + b, L1["ovlo_col"][c] + b))
                        for b in range(int(L1["OVhi"][c])):
                            ovblk.append((L1["ovhi_off"][c] - L1["ov_base"]
                                          + b, L1["ovhi_col"][c] + b))
                        aggov = None
                        if ovblk:
                            pov = psum.tile([128, 128], F32, tag="pov",
                                            bufs=1)
                            for i, (grow, dcol) in enumerate(ovblk):
                                sel = spool.tile([128, 128], BF16, tag="sel",
                                                 name="sel")
                                nc.vector.tensor_tensor(
                                    out=sel[:],
                                    in0=ovdst_s[:, dcol:dcol + 1]
                                    .to_broadcast([128, 128]),
                                    in1=iota_s[:], op=AX.is_equal)
                                nc.tensor.matmul(
                                    pov[:], lhsT=gov[:, grow, :], rhs=sel[:],
                                    start=(i == 0), stop=(i == len(ovblk) - 1))
                            aggov = spool.tile([128, 128], BF16, tag="aggov")
                            nc.scalar.activation(aggov[:], pov[:], ACTF.Copy)

                        hp = psum.tile([64, 128], F32, tag="hps", bufs=2)
                        nmm = (agg is not None) + (aggov is not None) + 1
                        i = 0
                        if agg is not None:
                            nc.tensor.matmul(hp[:], lhsT=wr1_s[:], rhs=agg,
                                             start=True, stop=(nmm == 1))
                            i += 1
                        if aggov is not None:
                            nc.tensor.matmul(hp[:], lhsT=wr1_s[:],
                                             rhs=aggov[:], start=(i == 0),
                                             stop=(i == nmm - 1))
                            i += 1
                        nc.tensor.matmul(hp[:], lhsT=wo1_s[:],
                                         rhs=xTown_s[:, sl], start=(i == 0),
                                         stop=True)
                        nc.scalar.activation(hT[0][:, sl], hp[:], ACTF.Relu)
                        store_z(0, c)
                        if c == C1 - 1:
                            kick_ag(1, 0, C1)

                # ============ Layers 2/3: fp32 z-row gathers ============
                for L, li in [(L2, 1), (L3, 2)]:
                    if dbg_stage < li + 1:
                        break
                    tab = z_tab[li]
                    tab_rows = [None, p.T1_rows, p.T2_rows][li]
                    wroot = w64["W_root2"] if li == 1 else w64["W_root3"]
                    for grp in L["groups"]:
                        rows_lo = sum(int(L["Rlo"][c] + L["OVlo"][c])
                                      for c in grp)
                        rows_hi = sum(int(L["Rhi"][c] + L["OVhi"][c])
                                      for c in grp)
                        rows = rows_lo + rows_hi
                        g = None
                        if rows:
                            g = gpool.tile([128, rows, 64], F32, tag="G2",
                                           name="G2")
                            if rows_lo:
                                gath(g[:, 0:rows_lo, :],
                                     tab[0:min(LO, tab_rows), :],
                                     L["lo_off"][grp[0]], rows_lo, 64, False)
                            if rows_hi:
                                gath(g[:, rows_lo:rows, :],
                                     tab[LO:tab_rows, :],
                                     L["hi_off"][grp[0]], rows_hi, 64, False)

                        for c in grp:
                            sl = slice(c * 128, (c + 1) * 128)
                            rlo = int(L["Rlo"][c])
                            rhi = int(L["Rhi"][c])

                            def fold(a, r):
                                k = r
                                while k > 1:
                                    m = (k + 1) // 2
                                    cnt = k - m
                                    nc.vector.tensor_tensor(
                                        out=g[:, a:a + cnt, :],
                                        in0=g[:, a:a + cnt, :],
                                        in1=g[:, a + m:a + k, :], op=AX.add)
                                    k = m

                            la = L["lo_off"][c] - L["lo_off"][grp[0]]
                            ha = rows_lo + (L["hi_off"][c]
                                            - L["hi_off"][grp[0]])
                            if rlo:
                                fold(la, rlo)
                            if rhi:
                                fold(ha, rhi)
                            if rlo and rhi:
                                nc.vector.tensor_tensor(
                                    out=g[:, la, :], in0=g[:, la, :],
                                    in1=g[:, ha, :], op=AX.add)
                            agg = (g[:, la, :] if rlo else
                                   (g[:, ha, :] if rhi else None))

                            ovblk = []
                            for b in range(int(L["OVlo"][c])):
                                grow = (L["ovlo_off"][c]
                                        - L["lo_off"][grp[0]] + b)
                                ovblk.append((grow, L["ovlo_col"][c] + b))
                            for b in range(int(L["OVhi"][c])):
                                grow = (rows_lo + L["ovhi_off"][c]
                                        - L["hi_off"][grp[0]] + b)
                                ovblk.append((grow, L["ovhi_col"][c] + b))

                            ap = psum.tile([64, 128], F32, tag="aggT",
                                           bufs=2)
                            nmm = (agg is not None) + len(ovblk) + 1
                            i = 0
                            if agg is not None:
                                nc.tensor.matmul(
                                    ap[:], lhsT=agg, rhs=ident[:],
                                    is_transpose=True, start=True,
                                    stop=False)
                                i = 1
                            for grow, dcol in ovblk:
                                sel = spool.tile([128, 128], F32,
                                                 tag="self", name="self")
                                nc.vector.tensor_tensor(
                                    out=sel[:],
                                    in0=ovdst_s[:, dcol:dcol + 1]
                                    .to_broadcast([128, 128]),
                                    in1=iota_s[:], op=AX.is_equal)
                                nc.tensor.matmul(
                                    ap[:], lhsT=g[:, grow, :], rhs=sel[:],
                                    start=(i == 0), stop=False)
                                i += 1
                            nc.tensor.matmul(ap[:], lhsT=wroot[:],
                                             rhs=hT[li - 1][:, sl],
                                             start=(i == 0), stop=True)
                            nc.scalar.activation(hT[li][:, sl], ap[:],
                                                 ACTF.Relu)
                            if li == 1:
                                store_z(1, c)
                                if c == C2 - 1:
                                    kick_ag(2, 0, C2)

                # ---- pool + fc + softmax ----
                if dbg_stage >= 3:
                    hm = spool.tile([64, C3 * 128], F32, tag="hm")
                    nc.vector.tensor_tensor(out=hm[:], in0=hT[2][:],
                                            in1=mask_s[:], op=AX.mult)
                    ppart = spool.tile([64, 1], F32, tag="ppart")
                    nc.vector.tensor_reduce(ppart[:], hm[:],
                                            axis=mybir.AxisListType.X,
                                            op=AX.add)
                    nc.sync.dma_start(pool_in[:], ppart[:])
                    if not skip_collectives:
                        nc.gpsimd.collective_compute(
                            "AllReduce", AX.add, replica_groups=rg,
                            ins=[pool_in.opt()], outs=[pool_out.opt()])
                    pooled = spool.tile([64, 1], F32, tag="pooled")
                    nc.sync.dma_start(pooled[:], pool_out[:])
                    mean_s = spool.tile([64, 1], F32, tag="mean")
                    nc.vector.tensor_scalar_mul(mean_s[:], pooled[:],
                                                1.0 / max(p.n0, 1))
                    lg_p = psum.tile([1, 10], F32, tag="lg")
                    nc.tensor.matmul(lg_p[:], lhsT=mean_s[:], rhs=wfc_s[:],
                                     start=True, stop=True)
                    logits = spool.tile([1, 10], F32, tag="logits")
                    nc.vector.tensor_tensor(out=logits[:], in0=lg_p[:],
                                            in1=bfc_s[:], op=AX.add)
                    mx = spool.tile([1, 1], F32, tag="mx")
                    nc.vector.tensor_reduce(mx[:], logits[:],
                                            axis=mybir.AxisListType.X,
                                            op=AX.max)
                    nmx = spool.tile([1, 1], F32, tag="nmx")
                    nc.vector.tensor_scalar_mul(nmx[:], mx[:], -1.0)
                    es = spool.tile([1, 10], F32, tag="es")
                    nc.scalar.activation(es[:], logits[:], ACTF.Exp,
                                         bias=nmx[:, 0:1])
                    ssum = spool.tile([1, 1], F32, tag="ssum")
                    nc.vector.tensor_reduce(ssum[:], es[:],
                                            axis=mybir.AxisListType.X,
                                            op=AX.add)
                    inv = spool.tile([1, 1], F32, tag="inv")
                    nc.vector.reciprocal(inv[:], ssum[:])
                    probs_s = spool.tile([1, 10], F32, tag="probs")
                    nc.vector.tensor_scalar_mul(probs_s[:], es[:],
                                                inv[:, 0:1])
                    nc.sync.dma_start(out_d[:], probs_s[:])
                else:
                    probs_dbg = spool.tile([1, 10], F32, tag="probs_dbg")
                    nc.vector.memset(probs_dbg[:], 0.5)
                    nc.sync.dma_start(out_d[:], probs_dbg[:])

    nc.compile()
    return nc


# ----------------------------------------------------------------------------
# Entry point
# ----------------------------------------------------------------------------

def _prep(inputs):
    x = np.ascontiguousarray(np.asarray(inputs["x"], np.float32))
    edge_index = np.asarray(inputs["edge_index"])
    batch = np.asarray(inputs["batch"])
    W = {k: np.ascontiguousarray(np.asarray(inputs[k], np.float32))
         for k in ["W_rel1", "W_root1", "W_rel2", "W_root2",
                   "W_rel3", "W_root3", "W_fc", "b_fc"]}
    p = build_plan(x, edge_index, batch)
    nc = build_program(p, W)
    return nc, _in_maps(p, W)


def _in_maps(p, W):
    import ml_dtypes
    bf = ml_dtypes.bfloat16
    in_maps = []
    for k in range(NCORES):
        im = {
            "x_tab": p.x_tab, "xTown": p.xTown[k], "idx": p.idx[k],
            "mask": p.mask[k], "ovdst": p.ovdst[k],
            "bW_rel1": W["W_rel1"].astype(bf),
            "bW_root1": W["W_root1"].astype(bf),
            "bW_rel2": W["W_rel2"].astype(bf),
            "bW_root2": W["W_root2"].astype(bf),
            "bW_rel3": W["W_rel3"].astype(bf),
            "bW_root3": W["W_root3"].astype(bf),
            "W_fc": W["W_fc"], "b_fc": W["b_fc"].reshape(1, 10),
            "iota": np.tile(np.arange(128, dtype=np.float32), (128, 1)),
        }
        in_maps.append(im)
    return in_maps


def kernel(**inputs) -> np.ndarray:
    nc, in_maps = _prep(inputs)
    res = run_bass_kernel_spmd(nc, in_maps, list(range(NCORES)))
    return np.asarray(res.results[0]["probs"]).reshape(10).astype(np.float32)
